# revision 1
# baseline (speedup 1.0000x reference)
# Chunked causal self-attention (Megalodon-style, chunk=2048) on 8 Trainium2
# NeuronCores via Bass/Tile.
#
# Problem (hardcoded): q,k,v (2, 4096, 16, 128) fp32, RoPE(10000) on q,k,
# per-chunk causal softmax(QK^T)V with scale 1.0.
#
# Sharding: 64 independent (batch, chunk, head) attention units of size
# (2048 x 2048 x 128); 8 units per core (4 (b,h) pairs x 2 chunks).
#
# Per-unit device pipeline:
#   DMA q,k (fp32, pre-transposed host layout) -> RoPE on DVE/ACT (3 TT passes)
#   -> PE transpose to [d, pos] (f32r)  -> S^T = K^T.T @ Q^T  (f32r matmuls)
#   -> +mask matmul on diagonal 128-blocks (bf16) -> ACT exp -> probs bf16
#   -> ones-matmul (denominators, replicated across partitions)
#   -> O^T = V.T-free accumulate (lhsT=V bf16, rhs=probs bf16)
#   -> recip_approx(denom) * O^T on DVE -> DMA out O^T (host transposes back).
import numpy as np
import ml_dtypes

B, T, H, DH, DV = 2, 4096, 16, 128, 128
CHUNK = 2048
NB = CHUNK // 128          # 16 key blocks per chunk
N_CORES = 8
UNITS = 8                  # (b,h) pairs per core * 2 chunks
BH_PER_CORE = (B * H) // N_CORES   # 4
ROPE_BASE = 10000.0
NEG = -1e30
QH = 1024                  # q-half width processed per pass (PSUM budget)

_RUNTIME = {}


def _build_program(reps=1):
    import concourse.tile as tile
    import concourse.mybir as mybir
    from concourse import bacc

    f32 = mybir.dt.float32
    f32r = mybir.dt.float32r
    bf16 = mybir.dt.bfloat16
    Exp = mybir.ActivationFunctionType.Exp

    nc = bacc.Bacc("TRN2", target_bir_lowering=False, debug=False,
                   num_devices=N_CORES)

    qc = nc.dram_tensor("qc", [UNITS, 128, CHUNK], f32, kind="ExternalInput").ap()
    kc = nc.dram_tensor("kc", [UNITS, 128, CHUNK], f32, kind="ExternalInput").ap()
    vc = nc.dram_tensor("vc", [UNITS, 128, CHUNK], bf16, kind="ExternalInput").ap()
    cosf = nc.dram_tensor("cosf", [2, 128, CHUNK], f32, kind="ExternalInput").ap()
    sinf = nc.dram_tensor("sinf", [2, 128, CHUNK], f32, kind="ExternalInput").ap()
    ident = nc.dram_tensor("ident", [128, 128], f32, kind="ExternalInput").ap()
    ident16 = nc.dram_tensor("ident16", [128, 128], bf16, kind="ExternalInput").ap()
    mask16 = nc.dram_tensor("mask16", [128, 128], bf16, kind="ExternalInput").ap()
    outT = nc.dram_tensor("outT", [UNITS, 128, CHUNK], f32, kind="ExternalOutput").ap()

    with tile.TileContext(nc) as tc:
        with tc.tile_pool(name="const", bufs=1) as cpool, \
             tc.tile_pool(name="work", bufs=2) as wpool, \
             tc.tile_pool(name="scratch", bufs=1) as spool, \
             tc.tile_pool(name="qkT", bufs=4) as tpool, \
             tc.tile_pool(name="probs", bufs=8) as ppool, \
             tc.tile_pool(name="psum", bufs=2, space="PSUM") as pspool, \
             tc.tile_pool(name="psumO", bufs=1, space="PSUM") as popool, \
             tc.tile_pool(name="psumD", bufs=1, space="PSUM") as pdpool:

            tcos = cpool.tile([128, 2 * CHUNK], f32, tag="tcos")
            tsin = cpool.tile([128, 2 * CHUNK], f32, tag="tsin")
            tid = cpool.tile([128, 128], f32, tag="tid")
            tidr = cpool.tile([128, 128], f32r, tag="tidr")
            tid16 = cpool.tile([128, 128], bf16, tag="tid16")
            tmask = cpool.tile([128, 128], bf16, tag="tmask")
            tones = cpool.tile([128, 128], bf16, tag="tones")
            for ch in range(2):
                nc.gpsimd.dma_start(out=tcos[:, ch * CHUNK:(ch + 1) * CHUNK], in_=cosf[ch])
                nc.gpsimd.dma_start(out=tsin[:, ch * CHUNK:(ch + 1) * CHUNK], in_=sinf[ch])
            nc.gpsimd.dma_start(out=tid[:], in_=ident[:])
            nc.gpsimd.dma_start(out=tid16[:], in_=ident16[:])
            nc.gpsimd.dma_start(out=tmask[:], in_=mask16[:])
            nc.gpsimd.memset(tones[:], 1.0)
            nc.vector.tensor_copy(tidr[:], tid[:])

            def load_rope(u):
                """DMA q,k,v of unit u + RoPE on DVE. Returns rope outputs
                (t1 tiles) + v tile."""
                ch = u % 2
                cosv = tcos[:, ch * CHUNK:(ch + 1) * CHUNK]
                sinv = tsin[:, ch * CHUNK:(ch + 1) * CHUNK]
                s4 = sinv.rearrange("p (b two d) -> p b two d", two=2, d=64)
                t1s = []
                for src in (qc, kc):
                    raw = wpool.tile([128, CHUNK], f32, tag="raw")
                    t1 = wpool.tile([128, CHUNK], f32r, tag="t1")
                    t2 = spool.tile([128, CHUNK], f32, tag="t2")
                    nc.sync.dma_start(out=raw[:], in_=src[u])
                    r4 = raw[:].rearrange("p (b two d) -> p b two d", two=2, d=64)
                    o4 = t2[:].rearrange("p (b two d) -> p b two d", two=2, d=64)
                    nc.any.tensor_mul(t1[:], raw[:], cosv)
                    nc.any.tensor_mul(o4[:, :, 0, :], r4[:, :, 1, :], s4[:, :, 0, :])
                    nc.any.tensor_mul(o4[:, :, 1, :], r4[:, :, 0, :], s4[:, :, 1, :])
                    nc.any.tensor_add(t1[:], t1[:], t2[:])
                    t1s.append(t1)
                tv = wpool.tile([128, CHUNK], bf16, tag="tv")
                nc.sync.dma_start(out=tv[:], in_=vc[u])
                return t1s[0], t1s[1], tv

            def transposes(t1q, t1k, psum_pool, ptag):
                """PE-transpose rope outputs into [d, pos] f32r SBUF tiles."""
                outs = []
                for t1 in (t1q, t1k):
                    dstT = tpool.tile([128, CHUNK], f32r, tag="tT")
                    for half in range(2):
                        pst = psum_pool.tile([128, QH], f32r, tag=ptag)
                        for blk in range(8):
                            g = half * 8 + blk
                            nc.tensor.transpose(
                                pst[:, blk * 128:(blk + 1) * 128],
                                t1[:, g * 128:(g + 1) * 128], tidr[:])
                        nc.scalar.copy(dstT[:, half * QH:(half + 1) * QH],
                                       pst[:])
                    outs.append(dstT)
                return outs[0], outs[1]

            def attention_half(u, hf, tqt, tkt, tv, after_first_row=None):
                jmax = 8 * hf + 7
                psO = popool.tile([128, QH], f32, tag="psO")
                psD = pdpool.tile([128, QH], f32, tag="psD")
                pending = []

                def emit_consumers(j, oj, probs):
                    # psD chunks first, then psO chunks: keeps the stationary
                    # operand (ones / V_j) constant across consecutive matmuls
                    for ps, lhsT in ((psD, tones[:]),
                                     (psO, tv[:, j * 128:(j + 1) * 128])):
                        for s in (0, 1):
                            lo, hi = max(oj, 512 * s), 512 * (s + 1)
                            if lo >= hi:
                                continue
                            last = (j == min(jmax, 8 * hf + 4 * s + 3))
                            nc.tensor.matmul(ps[:, lo:hi], lhsT=lhsT,
                                             rhs=probs[:, lo:hi],
                                             start=(j == 0), stop=last)

                for j in range(jmax + 1):
                    oj = max(0, 128 * j - QH * hf)
                    diag = (j >= 8 * hf)
                    psS = pspool.tile([128, QH], f32, tag="psS")
                    for s in (0, 1):
                        lo, hi = max(oj, 512 * s), 512 * (s + 1)
                        if lo >= hi:
                            continue
                        in_diag_bank = diag and (oj >= 512 * s) and (oj < hi)
                        nc.tensor.matmul(
                            psS[:, lo:hi],
                            lhsT=tkt[:, j * 128:(j + 1) * 128],
                            rhs=tqt[:, hf * QH + lo: hf * QH + hi],
                            start=True, stop=not in_diag_bank)
                        if in_diag_bank:
                            nc.tensor.matmul(
                                psS[:, oj:oj + 128], lhsT=tid16[:],
                                rhs=tmask[:], start=False, stop=True,
                                skip_group_check=True)
                    probs = ppool.tile([128, QH], bf16, tag="probs")
                    nc.scalar.activation(probs[:, oj:QH], psS[:, oj:QH], Exp)
                    pending.append((j, oj, probs))
                    if len(pending) > 3:
                        emit_consumers(*pending.pop(0))
                    if j == 0 and after_first_row is not None:
                        after_first_row()
                while pending:
                    emit_consumers(*pending.pop(0))

                rec = wpool.tile([128, QH], f32, tag="rec")
                osb = wpool.tile([128, QH], f32, tag="osb")
                for s in (0, 1):
                    sl = slice(512 * s, 512 * (s + 1))
                    nc.vector.reciprocal_approx_fast(out=rec[:, sl], in_=psD[:, sl])
                    nc.any.tensor_mul(osb[:, sl], psO[:, sl], rec[:, sl])
                    nc.sync.dma_start(
                        out=outT[u, :, hf * QH + 512 * s: hf * QH + 512 * (s + 1)],
                        in_=osb[:, sl])

            r = load_rope(0)
            cur = transposes(r[0], r[1], pspool, "psS") + (r[2],)
            for _rep in range(reps):
                for u in range(UNITS):
                    # prefetch next unit (wrapping into the next rep):
                    # DMA + rope before this unit's attention
                    has_next = (u + 1 < UNITS) or (_rep + 1 < reps)
                    if has_next:
                        nxt = load_rope((u + 1) % UNITS)
                    attention_half(u, 0, cur[0], cur[1], cur[2])
                    # transpose next unit's rope output inside half 1, after
                    # its first S row (borrows the psD slot, which frees once
                    # half 0's reciprocal has read it)
                    holder = {}
                    hook = None
                    if has_next:
                        def hook(nxt=nxt, holder=holder):
                            holder["T"] = transposes(nxt[0], nxt[1], pdpool, "psD")
                    attention_half(u, 1, cur[0], cur[1], cur[2],
                                   after_first_row=hook)
                    if has_next:
                        nxt_T = holder["T"]
                        cur = (nxt_T[0], nxt_T[1], nxt[2])
    nc.compile()
    return nc


def _make_runner(nc):
    """Cached PJRT runner (clone of bass2jax.run_bass_via_pjrt multi-core
    path, but keeping the jitted callable so repeat calls don't recompile)."""
    import jax
    import concourse.mybir as mybir
    from concourse import bass2jax
    from jax.sharding import Mesh, PartitionSpec
    from jax.experimental.shard_map import shard_map

    bass2jax.install_neuronx_cc_hook()

    partition_name = (nc.partition_id_tensor.name
                      if nc.partition_id_tensor else None)
    in_names, out_names, out_avals, zero_outs = [], [], [], []
    for alloc in nc.m.functions[0].allocations:
        if not isinstance(alloc, mybir.MemoryLocationSet):
            continue
        name = alloc.memorylocations[0].name
        if alloc.kind == "ExternalInput":
            if name != partition_name:
                in_names.append(name)
        elif alloc.kind == "ExternalOutput":
            shape = tuple(alloc.tensor_shape)
            dtype = mybir.dt.np(alloc.dtype)
            out_names.append(name)
            out_avals.append(jax.core.ShapedArray(shape, dtype))
            zero_outs.append(np.zeros(shape, dtype))
    n_params = len(in_names)
    n_outs = len(out_avals)
    all_names = in_names + out_names
    if partition_name is not None:
        all_names = all_names + [partition_name]
    donate = tuple(range(n_params, n_params + n_outs))

    def _body(*args):
        operands = list(args)
        if partition_name is not None:
            operands.append(bass2jax.partition_id_tensor())
        outs = bass2jax._bass_exec_p.bind(
            *operands, out_avals=tuple(out_avals), in_names=tuple(all_names),
            out_names=tuple(out_names), lowering_input_output_aliases=(),
            sim_require_finite=True, sim_require_nnan=True, nc=nc)
        return tuple(outs)

    devices = jax.devices()[:N_CORES]
    mesh = Mesh(np.asarray(devices), ("core",))
    sharded = jax.jit(
        shard_map(_body, mesh=mesh,
                  in_specs=(PartitionSpec("core"),) * (n_params + n_outs),
                  out_specs=(PartitionSpec("core"),) * n_outs,
                  check_rep=False),
        donate_argnums=donate, keep_unused=True)

    def run(in_maps):
        concat_in = [np.concatenate([m[name] for m in in_maps], axis=0)
                     for name in in_names]
        concat_zero = [np.concatenate([z] * N_CORES, axis=0) for z in zero_outs]
        outs = sharded(*concat_in, *concat_zero)
        outs = [np.asarray(o) for o in outs]
        res = []
        for c in range(N_CORES):
            d = {}
            for i, name in enumerate(out_names):
                per = outs[i].shape[0] // N_CORES
                d[name] = outs[i][c * per:(c + 1) * per]
            res.append(d)
        return res

    return run


def _rope_tables(start_index):
    half = DH // 2
    inv_freq = np.exp(np.arange(half, dtype=np.float64) *
                      (-(np.log(ROPE_BASE) / half)))
    pos = np.arange(T, dtype=np.float64) + float(start_index)
    ang = pos[:, None] * inv_freq[None, :]          # (T, 64)
    cos = np.cos(ang)
    sin = np.sin(ang)
    cosfull = np.concatenate([cos, cos], axis=1)    # (T, 128)
    sinfull = np.concatenate([-sin, sin], axis=1)
    # (T,128) -> (2, 16, 128, 128)[c, pb, p, d] -> (2, 128, 16*128)
    def lay(x):
        x = x.reshape(2, NB, 128, DH).transpose(0, 2, 1, 3).reshape(2, 128, CHUNK)
        return np.ascontiguousarray(x, dtype=np.float32)
    return lay(cosfull), lay(sinfull)


def _shard_inputs(q, k, v, start_index):
    q = np.asarray(q, dtype=np.float32)
    k = np.asarray(k, dtype=np.float32)
    v = np.asarray(v, dtype=np.float32)
    cosf, sinf = _rope_tables(start_index)
    ident = np.eye(128, dtype=np.float32)
    i = np.arange(128)
    mask16 = np.where(i[:, None] <= i[None, :], 0.0, NEG).astype(ml_dtypes.bfloat16)

    # layout per unit: [p, blk*128+d] with pos = blk*128 + p
    def lay(x):  # (2048, 128) -> (128, 2048)
        return x.reshape(NB, 128, DH).transpose(1, 0, 2).reshape(128, CHUNK)

    in_maps = []
    for c in range(N_CORES):
        qu = np.empty((UNITS, 128, CHUNK), np.float32)
        ku = np.empty((UNITS, 128, CHUNK), np.float32)
        vu = np.empty((UNITS, 128, CHUNK), ml_dtypes.bfloat16)
        for ubh in range(BH_PER_CORE):
            bh = c * BH_PER_CORE + ubh
            b, h = bh // H, bh % H
            for ch in range(2):
                u = ubh * 2 + ch
                sl = slice(ch * CHUNK, (ch + 1) * CHUNK)
                qu[u] = lay(q[b, sl, h, :])
                ku[u] = lay(k[b, sl, h, :])
                vu[u] = lay(v[b, sl, h, :]).astype(ml_dtypes.bfloat16)
        in_maps.append({"qc": qu, "kc": ku, "vc": vu, "cosf": cosf,
                        "sinf": sinf, "ident": ident,
                        "ident16": ident.astype(ml_dtypes.bfloat16),
                        "mask16": mask16})
    return in_maps


def _gather_output(results):
    out = np.empty((B, T, H, DV), np.float32)
    for c in range(N_CORES):
        oT = results[c]["outT"]        # (UNITS, 128 dv, 2048 q)
        for ubh in range(BH_PER_CORE):
            bh = c * BH_PER_CORE + ubh
            b, h = bh // H, bh % H
            for ch in range(2):
                u = ubh * 2 + ch
                out[b, ch * CHUNK:(ch + 1) * CHUNK, h, :] = oT[u].T
    return out


def get_runtime(reps=1):
    if reps not in _RUNTIME:
        nc = _build_program(reps)
        _RUNTIME[reps] = _make_runner(nc)
    return _RUNTIME[reps]


def kernel(q, k, v, start_index):
    run = get_runtime()
    in_maps = _shard_inputs(q, k, v, start_index)
    results = run(in_maps)
    return _gather_output(results)


if __name__ == "__main__":
    rng = np.random.default_rng(0)
    q = rng.standard_normal((B, T, H, DH)).astype(np.float32)
    k = rng.standard_normal((B, T, H, DH)).astype(np.float32)
    v = rng.standard_normal((B, T, H, DV)).astype(np.float32)
    out = kernel(q, k, v, 0)
    print("out", out.shape, out.dtype, np.abs(out).max())



# revision 3
# speedup vs baseline: 1.0986x; 1.0986x over previous
# Chunked causal self-attention (Megalodon-style, chunk=2048) on 8 Trainium2
# NeuronCores via Bass/Tile. v2: transposed host layout + swap-matmul RoPE,
# denominator via bf16 probs accumulation on DVE/Pool instead of per-block
# ones-matmuls on PE.
#
# Problem (hardcoded): q,k,v (2, 4096, 16, 128) fp32, RoPE(10000) on q,k,
# per-chunk causal softmax(QK^T)V with scale 1.0.
#
# Sharding: 64 independent (batch, chunk, head) attention units of size
# (2048 x 2048 x 128); 8 units per core (4 (b,h) pairs x 2 chunks).
#
# Per-unit device pipeline:
#   DMA qT,kT (fp32, host provides [d, pos] transposed layout) ->
#   RoPE in transposed layout: t = x*cosT (DVE), xs = x*sinT~ (Pool),
#   t2 = SWAP @ xs (PE matmul with 128x128 permutation), t += t2 (DVE)
#   -> S^T = K_j^T.T @ Q^T per 128-key block (f32r matmuls)
#   -> +mask matmul on diagonal 128-blocks (bf16) -> ACT exp -> probs bf16
#   -> O^T accumulate (lhsT=V_j bf16, rhs=probs)
#   -> denominator: probs accumulated elementwise in bf16 (DVE 2x / Pool
#      chains) + ones-matmul per chain; late (cheap) blocks as direct
#      ones-matmuls on PE
#   -> recip_approx_fast(psD) * psO on DVE -> DMA out O^T.
import numpy as np
import ml_dtypes

B, T, H, DH, DV = 2, 4096, 16, 128, 128
CHUNK = 2048
NB = CHUNK // 128          # 16 key blocks per chunk
N_CORES = 8
UNITS = 8                  # (b,h) pairs per core * 2 chunks
BH_PER_CORE = (B * H) // N_CORES   # 4
ROPE_BASE = 10000.0
NEG = -1e30
QH = 1024                  # q-half width processed per pass (PSUM budget)

# Per (hf, j) engine assignment for the denominator reduction.
#   "dve"/"pool": elementwise bf16 accumulate on that engine (chain)
#   "pe": direct ones-matmul into psD (use for late/cheap blocks only, so
#         psD's PSUM slot is allocated late; see px-ring comment below)
# j=0 must be "dve" (that chain provides full [0:QH) coverage for psD).
# Pool is slow per column: give it early blocks so its chain finishes
# mid-half and never sits on the half's critical tail.
ACC_ASSIGN = {}
for _j in range(8):
    ACC_ASSIGN[(0, _j)] = "dve" if _j < 4 else "pe"
for _j in range(16):
    if _j >= 9:
        ACC_ASSIGN[(1, _j)] = "pe"
    elif _j in (1, 2, 3):
        ACC_ASSIGN[(1, _j)] = "pool"
    else:
        ACC_ASSIGN[(1, _j)] = "dve"

_RUNTIME = {}


def _build_program(reps=1):
    import concourse.tile as tile
    import concourse.mybir as mybir
    from concourse import bacc

    f32 = mybir.dt.float32
    f32r = mybir.dt.float32r
    f16 = mybir.dt.float16
    bf16 = mybir.dt.bfloat16
    Exp = mybir.ActivationFunctionType.Exp

    nc = bacc.Bacc("TRN2", target_bir_lowering=False, debug=False,
                   num_devices=N_CORES)

    qc = nc.dram_tensor("qc", [UNITS, 128, CHUNK], f16, kind="ExternalInput").ap()
    kc = nc.dram_tensor("kc", [UNITS, 128, CHUNK], f16, kind="ExternalInput").ap()
    vc = nc.dram_tensor("vc", [UNITS, 128, CHUNK], bf16, kind="ExternalInput").ap()
    cosf = nc.dram_tensor("cosf", [2, 128, CHUNK], f32, kind="ExternalInput").ap()
    sinf = nc.dram_tensor("sinf", [2, 128, CHUNK], f32, kind="ExternalInput").ap()
    swapm = nc.dram_tensor("swapm", [128, 128], f32, kind="ExternalInput").ap()
    ident16 = nc.dram_tensor("ident16", [128, 128], bf16, kind="ExternalInput").ap()
    mask16 = nc.dram_tensor("mask16", [128, 128], bf16, kind="ExternalInput").ap()
    outT = nc.dram_tensor("outT", [UNITS, 128, CHUNK], bf16, kind="ExternalOutput").ap()

    with tile.TileContext(nc) as tc:
        with tc.tile_pool(name="const", bufs=1) as cpool, \
             tc.tile_pool(name="work", bufs=2) as wpool, \
             tc.tile_pool(name="scratch", bufs=2) as spool, \
             tc.tile_pool(name="probs", bufs=12) as ppool, \
             tc.tile_pool(name="accp", bufs=4) as apool, \
             tc.tile_pool(name="psum", bufs=2, space="PSUM") as pspool, \
             tc.tile_pool(name="psumO", bufs=2, space="PSUM") as popool, \
             tc.tile_pool(name="psumX", bufs=2, space="PSUM") as pxpool:

            tcos = cpool.tile([128, 2 * CHUNK], f32, tag="tcos")
            tsin = cpool.tile([128, 2 * CHUNK], f32, tag="tsin")
            tsw = cpool.tile([128, 128], f32, tag="tsw")
            tswr = cpool.tile([128, 128], f32r, tag="tswr")
            tid16 = cpool.tile([128, 128], bf16, tag="tid16")
            tmask = cpool.tile([128, 128], bf16, tag="tmask")
            tones = cpool.tile([128, 128], bf16, tag="tones")
            for ch in range(2):
                nc.gpsimd.dma_start(out=tcos[:, ch * CHUNK:(ch + 1) * CHUNK], in_=cosf[ch])
                nc.gpsimd.dma_start(out=tsin[:, ch * CHUNK:(ch + 1) * CHUNK], in_=sinf[ch])
            nc.gpsimd.dma_start(out=tsw[:], in_=swapm[:])
            nc.gpsimd.dma_start(out=tid16[:], in_=ident16[:])
            nc.gpsimd.dma_start(out=tmask[:], in_=mask16[:])
            nc.gpsimd.memset(tones[:], 1.0)
            nc.vector.tensor_copy(tswr[:], tsw[:])

            def load_rope(u):
                """DMA q,k,v of unit u (transposed [d,pos] layout) + RoPE.
                cos-mul + final add on DVE, sin-mul on Pool, partition swap
                via PE matmul with the permutation matrix."""
                ch = u % 2
                cosv = tcos[:, ch * CHUNK:(ch + 1) * CHUNK]
                sinv = tsin[:, ch * CHUNK:(ch + 1) * CHUNK]
                outs = []
                for src, tg in ((qc, "tq"), (kc, "tk")):
                    raw = spool.tile([128, CHUNK], f16, tag="raw", name="raw")
                    xs = spool.tile([128, CHUNK], f32r, tag="xs", name="xs")
                    t = wpool.tile([128, CHUNK], f32r, tag=tg, name=tg)
                    nc.sync.dma_start(out=raw[:], in_=src[u])
                    nc.vector.tensor_mul(t[:], raw[:], cosv)
                    nc.gpsimd.tensor_mul(xs[:], raw[:], sinv)
                    for seg in range(4):
                        sl = slice(512 * seg, 512 * (seg + 1))
                        ps = pxpool.tile([128, 512], f32, tag="px", name="px")
                        nc.tensor.matmul(ps[:], lhsT=tswr[:], rhs=xs[:, sl],
                                         start=True, stop=True)
                        nc.vector.tensor_add(t[:, sl], t[:, sl], ps[:])
                    outs.append(t)
                tv = wpool.tile([128, CHUNK], bf16, tag="tv", name="tv")
                nc.sync.dma_start(out=tv[:], in_=vc[u])
                return outs[0], outs[1], tv

            def attention_half(u, hf, tqt, tkt, tv):
                jmax = 8 * hf + 7
                late0 = 4 + 8 * hf      # first j with oj >= 512 (bank1 only)
                psO = {}                # s -> [128,512] tile (per-bank ring)
                chains = {}             # engine -> (acc_tile, oj_first)
                direct = []             # (j, oj, probs): "pe" blocks, D-mms
                pending = []            # deferred to the drain

                def get_psO(s):
                    if s not in psO:
                        psO[s] = popool.tile([128, 512], f32, tag="psO",
                                             name="psO")
                    return psO[s]

                def emit_consumers(j, oj, probs):
                    eng = ACC_ASSIGN[(hf, j)]
                    if eng == "pe":
                        direct.append((j, oj, probs))
                    elif eng in chains:
                        acc = chains[eng][0]
                        op = nc.vector if eng == "dve" else nc.gpsimd
                        op.tensor_add(acc[:, oj:QH], acc[:, oj:QH],
                                      probs[:, oj:QH])
                    else:
                        acc = apool.tile([128, QH], bf16, tag="acc", name="acc")
                        chains[eng] = (acc, oj)
                        op = nc.vector if eng == "dve" else nc.gpsimd
                        op.tensor_copy(acc[:, oj:QH], probs[:, oj:QH])
                    for s in (0, 1):
                        lo, hi = max(oj, 512 * s), 512 * (s + 1)
                        if lo >= hi:
                            continue
                        last = (j == min(jmax, 8 * hf + 4 * s + 3))
                        nc.tensor.matmul(get_psO(s)[:, lo - 512 * s:hi - 512 * s],
                                         lhsT=tv[:, j * 128:(j + 1) * 128],
                                         rhs=probs[:, lo:hi],
                                         start=(j == 0), stop=last)

                for j in range(jmax + 1):
                    oj = max(0, 128 * j - QH * hf)
                    diag = (j >= 8 * hf)
                    if j >= late0:
                        # bank1-only block: [128,512] tile from the px ring
                        # keeps the main psS ring recycling early blocks so
                        # the next half's S-matmuls aren't throttled by the
                        # ACT exp backlog at the half boundary.
                        ps = pxpool.tile([128, 512], f32, tag="px",
                                         name="psSx")
                        nc.tensor.matmul(
                            ps[:, oj - 512:512],
                            lhsT=tkt[:, j * 128:(j + 1) * 128],
                            rhs=tqt[:, hf * QH + oj: hf * QH + QH],
                            start=True, stop=not diag)
                        if diag:
                            nc.tensor.matmul(
                                ps[:, oj - 512:oj - 512 + 128], lhsT=tid16[:],
                                rhs=tmask[:], start=False, stop=True,
                                skip_group_check=True)
                        src = ps[:, oj - 512:512]
                    else:
                        psS = pspool.tile([128, QH], f32, tag="psS", name="psS")
                        for s in (0, 1):
                            lo, hi = max(oj, 512 * s), 512 * (s + 1)
                            if lo >= hi:
                                continue
                            in_diag_bank = diag and (oj >= 512 * s) and (oj < hi)
                            nc.tensor.matmul(
                                psS[:, lo:hi],
                                lhsT=tkt[:, j * 128:(j + 1) * 128],
                                rhs=tqt[:, hf * QH + lo: hf * QH + hi],
                                start=True, stop=not in_diag_bank)
                            if in_diag_bank:
                                nc.tensor.matmul(
                                    psS[:, oj:oj + 128], lhsT=tid16[:],
                                    rhs=tmask[:], start=False, stop=True,
                                    skip_group_check=True)
                        src = psS[:, oj:QH]
                    probs = ppool.tile([128, QH], bf16, tag="probs", name="probs")
                    nc.scalar.activation(probs[:, oj:QH], src, Exp)
                    pending.append((j, oj, probs))
                    if len(pending) > 3:
                        emit_consumers(*pending.pop(0))
                while pending:
                    emit_consumers(*pending.pop(0))

                # Denominator matmuls, per psD bank: the "dve" chain (which
                # contains j=0, so covers the full bank) goes first with
                # start=True, clearing stale has_written bits; then "pool";
                # then the direct blocks in j order (last one stops the
                # group). Bank0's writers all retire by ~exp(late0-1), so
                # recip/mul/psO-release for bank0 happen mid-half.
                chain_list = [e for e in ("dve", "pool") if e in chains]
                rec = spool.tile([128, QH], f32, tag="rec", name="rec")
                osb = spool.tile([128, QH], bf16, tag="osb", name="osb")
                for s in (0, 1):
                    lo_b, hi_b = 512 * s, 512 * (s + 1)
                    writers = []
                    for eng in chain_list:
                        acc, oj0 = chains[eng]
                        if max(oj0, lo_b) < hi_b:
                            writers.append((max(oj0, lo_b), acc))
                    for (j, oj, probs) in direct:
                        if max(oj, lo_b) < hi_b:
                            writers.append((max(oj, lo_b), probs))
                    psD = pxpool.tile([128, 512], f32, tag="px", name="psD")
                    for wi, (lo, rhs_t) in enumerate(writers):
                        nc.tensor.matmul(psD[:, lo - lo_b:512],
                                         lhsT=tones[:], rhs=rhs_t[:, lo:hi_b],
                                         start=(wi == 0),
                                         stop=(wi == len(writers) - 1))
                    sl = slice(lo_b, hi_b)
                    nc.vector.reciprocal_approx_fast(out=rec[:, sl],
                                                     in_=psD[:])
                    nc.vector.tensor_mul(osb[:, sl], psO[s][:], rec[:, sl])
                nc.sync.dma_start(
                    out=outT[u, :, hf * QH:(hf + 1) * QH], in_=osb[:])

            cur = load_rope(0)
            for _rep in range(reps):
                for u in range(UNITS):
                    has_next = (u + 1 < UNITS) or (_rep + 1 < reps)
                    attention_half(u, 0, cur[0], cur[1], cur[2])
                    if has_next:
                        nxt = load_rope((u + 1) % UNITS)
                    attention_half(u, 1, cur[0], cur[1], cur[2])
                    if has_next:
                        cur = nxt
    nc.compile()
    return nc


def _make_runner(nc):
    """Cached PJRT runner (clone of bass2jax.run_bass_via_pjrt multi-core
    path, but keeping the jitted callable so repeat calls don't recompile)."""
    import jax
    import concourse.mybir as mybir
    from concourse import bass2jax
    from jax.sharding import Mesh, PartitionSpec
    from jax.experimental.shard_map import shard_map

    bass2jax.install_neuronx_cc_hook()

    partition_name = (nc.partition_id_tensor.name
                      if nc.partition_id_tensor else None)
    in_names, out_names, out_avals, zero_outs = [], [], [], []
    for alloc in nc.m.functions[0].allocations:
        if not isinstance(alloc, mybir.MemoryLocationSet):
            continue
        name = alloc.memorylocations[0].name
        if alloc.kind == "ExternalInput":
            if name != partition_name:
                in_names.append(name)
        elif alloc.kind == "ExternalOutput":
            shape = tuple(alloc.tensor_shape)
            dtype = mybir.dt.np(alloc.dtype)
            out_names.append(name)
            out_avals.append(jax.core.ShapedArray(shape, dtype))
            zero_outs.append(np.zeros(shape, dtype))
    n_params = len(in_names)
    n_outs = len(out_avals)
    all_names = in_names + out_names
    if partition_name is not None:
        all_names = all_names + [partition_name]
    donate = tuple(range(n_params, n_params + n_outs))

    def _body(*args):
        operands = list(args)
        if partition_name is not None:
            operands.append(bass2jax.partition_id_tensor())
        outs = bass2jax._bass_exec_p.bind(
            *operands, out_avals=tuple(out_avals), in_names=tuple(all_names),
            out_names=tuple(out_names), lowering_input_output_aliases=(),
            sim_require_finite=True, sim_require_nnan=True, nc=nc)
        return tuple(outs)

    devices = jax.devices()[:N_CORES]
    mesh = Mesh(np.asarray(devices), ("core",))
    sharded = jax.jit(
        shard_map(_body, mesh=mesh,
                  in_specs=(PartitionSpec("core"),) * (n_params + n_outs),
                  out_specs=(PartitionSpec("core"),) * n_outs,
                  check_rep=False),
        donate_argnums=donate, keep_unused=True)

    def run(in_maps):
        concat_in = [np.concatenate([m[name] for m in in_maps], axis=0)
                     for name in in_names]
        concat_zero = [np.concatenate([z] * N_CORES, axis=0) for z in zero_outs]
        outs = sharded(*concat_in, *concat_zero)
        outs = [np.asarray(o) for o in outs]
        res = []
        for c in range(N_CORES):
            d = {}
            for i, name in enumerate(out_names):
                per = outs[i].shape[0] // N_CORES
                d[name] = outs[i][c * per:(c + 1) * per]
            res.append(d)
        return res

    return run


def _rope_tables(start_index):
    """Transposed-layout tables: cosT[c][d,p] = cos((c*2048+p+si)*f[d%64]);
    sinT~[c][d,p] = +sin(.*f[d]) for d<64, -sin(.*f[d-64]) for d>=64 (the
    sign/ordering expected by the swap-matmul formulation)."""
    half = DH // 2
    inv_freq = np.exp(np.arange(half, dtype=np.float64) *
                      (-(np.log(ROPE_BASE) / half)))
    pos = np.arange(T, dtype=np.float64) + float(start_index)
    ang = pos[:, None] * inv_freq[None, :]          # (T, 64)
    cos = np.cos(ang)
    sin = np.sin(ang)
    cosT = np.concatenate([cos, cos], axis=1).T     # (128, T)
    sinT = np.concatenate([sin, -sin], axis=1).T
    cosT = cosT.reshape(128, 2, CHUNK).transpose(1, 0, 2)
    sinT = sinT.reshape(128, 2, CHUNK).transpose(1, 0, 2)
    return (np.ascontiguousarray(cosT, dtype=np.float32),
            np.ascontiguousarray(sinT, dtype=np.float32))


def _shard_inputs(q, k, v, start_index):
    q = np.asarray(q, dtype=np.float32)
    k = np.asarray(k, dtype=np.float32)
    v = np.asarray(v, dtype=np.float32)
    cosT, sinT = _rope_tables(start_index)
    i = np.arange(128)
    mask16 = np.where(i[:, None] <= i[None, :], 0.0, NEG).astype(ml_dtypes.bfloat16)
    ident16 = np.eye(128, dtype=ml_dtypes.bfloat16)
    swapm = np.zeros((128, 128), np.float32)
    swapm[(i + 64) % 128, i] = 1.0

    # v layout per unit: [p, blk*128+d] with pos = blk*128 + p
    def layv(x):  # (2048, 128) -> (128, 2048)
        return x.reshape(NB, 128, DV).transpose(1, 0, 2).reshape(128, CHUNK)

    in_maps = []
    for c in range(N_CORES):
        qu = np.empty((UNITS, 128, CHUNK), np.float16)
        ku = np.empty((UNITS, 128, CHUNK), np.float16)
        vu = np.empty((UNITS, 128, CHUNK), ml_dtypes.bfloat16)
        for ubh in range(BH_PER_CORE):
            bh = c * BH_PER_CORE + ubh
            b, h = bh // H, bh % H
            for ch in range(2):
                u = ubh * 2 + ch
                sl = slice(ch * CHUNK, (ch + 1) * CHUNK)
                qu[u] = q[b, sl, h, :].T        # (128 d, 2048 pos)
                ku[u] = k[b, sl, h, :].T
                vu[u] = layv(v[b, sl, h, :]).astype(ml_dtypes.bfloat16)
        in_maps.append({"qc": qu, "kc": ku, "vc": vu, "cosf": cosT,
                        "sinf": sinT, "swapm": swapm,
                        "ident16": ident16, "mask16": mask16})
    return in_maps


def _gather_output(results):
    out = np.empty((B, T, H, DV), np.float32)
    for c in range(N_CORES):
        oT = results[c]["outT"]        # (UNITS, 128 dv, 2048 q)
        for ubh in range(BH_PER_CORE):
            bh = c * BH_PER_CORE + ubh
            b, h = bh // H, bh % H
            for ch in range(2):
                u = ubh * 2 + ch
                out[b, ch * CHUNK:(ch + 1) * CHUNK, h, :] = oT[u].T
    return out


def get_runtime(reps=1):
    if reps not in _RUNTIME:
        nc = _build_program(reps)
        _RUNTIME[reps] = _make_runner(nc)
    return _RUNTIME[reps]


def kernel(q, k, v, start_index):
    run = get_runtime()
    in_maps = _shard_inputs(q, k, v, start_index)
    results = run(in_maps)
    return _gather_output(results)


if __name__ == "__main__":
    rng = np.random.default_rng(0)
    q = rng.standard_normal((B, T, H, DH)).astype(np.float32)
    k = rng.standard_normal((B, T, H, DH)).astype(np.float32)
    v = rng.standard_normal((B, T, H, DV)).astype(np.float32)
    out = kernel(q, k, v, 0)
    print("out", out.shape, out.dtype, np.abs(out).max())
